# revision 7
# baseline (speedup 1.0000x reference)
"""BitLinear TRN2 kernel: y = x @ W(pweight,nweight)^T + bias.

Sharding: 8 cores = 4 token-shards x 2 out-feature-shards.
Per core: xt [2048 i, 4096 t] bf16 (token slice, host-transposed + cast),
pwt/nwt [512 i, 1024*4 (o,n)] bf16 (host-transposed so i is the partition
dim on device -> no PE transposes in weight prep).

Device pipeline (bf16 wire dtypes, fp32 PSUM accumulation):
  weights: DMA pwt/nwt -> ACT sigmoid -> DVE subtract -> DVE scale by
           c[n]=exps[n]*sigmoid(mask[n])*scale -> DVE reduce over n
           -> wpart [128, 4, 1024] bf16 -> AllGather over the 4 token
           shards (i-slices) -> wT [128 ip, 16 it, 1024 o] in SBUF.
  main:    per 128-token tile: lhsT = x-tile [128 i, 128 t] stationary,
           psum[t, o] += lhsT.T @ wT-slice over 16 i-tiles x 2 psum banks;
           DVE adds bias (host-replicated [128, 1024] tile) during
           PSUM->SBUF copy (bf16 out); DMA out y [t, o] bf16.

bias path: bit_ste is an exact identity on the reference's bias_raw values
(k/15 grid), computed host-side along with the tiny c vector.
"""

import numpy as np

import concourse.bass as bass
import concourse.mybir as mybir
import concourse.tile as tile
from concourse import bacc
from concourse.bass_utils import run_bass_kernel_spmd

N_CORES = 8
R, C = 4, 2  # token shards x out-feature shards
T, I, O, NB = 16384, 2048, 2048, 4
TQ, OC = T // R, O // C  # 4096 tokens, 1024 outs per core
P = 128
N_IT = I // P  # 16 i-tiles
ISH = I // R  # 512 i-rows of weight prep done locally per core
N_ITL = ISH // P  # 4 local i-tiles
WCOL = OC * NB  # 4096 flattened (o, n) columns of pwt/nwt
TSLAB = 512  # tokens per x slab (4 t-tiles)
N_SLAB = TQ // TSLAB
VPS = TSLAB // P  # t-tiles per slab
DT = mybir.dt.bfloat16

_BUILT = None


def _build_bass(reps=1, mode='full'):
    nc = bacc.Bacc("TRN2", debug=False, num_devices=N_CORES)

    xt_d = nc.dram_tensor("xt", [I, TQ], DT, kind="ExternalInput").ap()
    pw_d = nc.dram_tensor("pw", [ISH, WCOL], DT, kind="ExternalInput").ap()
    nw_d = nc.dram_tensor("nw", [ISH, WCOL], DT, kind="ExternalInput").ap()
    cv_d = nc.dram_tensor("cvec", [P, NB], DT, kind="ExternalInput").ap()
    bias_d = nc.dram_tensor("bias", [P, OC], mybir.dt.float32, kind="ExternalInput").ap()
    y_d = nc.dram_tensor("y", [TQ, OC], DT, kind="ExternalOutput").ap()

    with tile.TileContext(nc) as tc:
        with (
            tc.tile_pool(name="const", bufs=1) as const_pool,
            tc.tile_pool(name="wT", bufs=2) as wT_pool,
            tc.tile_pool(name="wpart", bufs=2) as wpart_pool,
            tc.tile_pool(name="dram", bufs=2, space="DRAM") as dram_pool,
            tc.tile_pool(name="wio", bufs=2) as wio_pool,
            tc.tile_pool(name="sig", bufs=2) as sig_pool,
            tc.tile_pool(name="soft", bufs=2) as soft_pool,
            tc.tile_pool(name="xs", bufs=2) as xs_pool,
            tc.tile_pool(name="yo", bufs=3) as yo_pool,
            tc.tile_pool(name="mm_ps", bufs=3, space="PSUM") as mm_ps,
        ):
            cv_sb = const_pool.tile([P, NB], DT)
            nc.sync.dma_start(cv_sb[:], cv_d[:])
            bias_sb = const_pool.tile([P, OC], mybir.dt.float32)
            nc.sync.dma_start(bias_sb[:], bias_d[:])

            for _rep in range(reps):
                # ---------------- weight stage (local i-slice) ----------------
                if mode != 'mm':
                    wpart = wpart_pool.tile([P, N_ITL, OC], DT, tag="wpart")
                    for itl in range(N_ITL):
                        irow = slice(itl * P, (itl + 1) * P)
                        for oh in range(2):  # half of (o, n) columns at a time
                            fcol = slice(oh * (WCOL // 2), (oh + 1) * (WCOL // 2))
                            ocol = slice(oh * (OC // 2), (oh + 1) * (OC // 2))
                            pwt = wio_pool.tile([P, WCOL // 2], DT, tag="pw")
                            nc.scalar.dma_start(pwt[:], pw_d[irow, fcol])
                            nwt = wio_pool.tile([P, WCOL // 2], DT, tag="nw")
                            nc.scalar.dma_start(nwt[:], nw_d[irow, fcol])
                            if mode == 'dma':
                                continue
                            sp = sig_pool.tile([P, WCOL // 2], DT, tag="sp")
                            nc.scalar.activation(
                                sp[:], pwt[:], mybir.ActivationFunctionType.Sigmoid
                            )
                            sn = sig_pool.tile([P, WCOL // 2], DT, tag="sn")
                            nc.scalar.activation(
                                sn[:], nwt[:], mybir.ActivationFunctionType.Sigmoid
                            )
                            soft = soft_pool.tile([P, WCOL // 2], DT, tag="soft")
                            nc.vector.tensor_sub(out=soft[:], in0=sp[:], in1=sn[:])
                            # scaled[i, o, n] = soft * c[n]; wpart[i, o] = sum_n
                            scaled = soft_pool.tile([P, WCOL // 2], DT, tag="scl")
                            nc.vector.tensor_tensor(
                                scaled[:].rearrange("p (o n) -> p o n", n=NB),
                                soft[:].rearrange("p (o n) -> p o n", n=NB),
                                cv_sb[:, None, :].to_broadcast((P, OC // 2, NB)),
                                mybir.AluOpType.mult,
                            )
                            with nc.allow_low_precision(
                                reason="4-term bf16 sum of bounded bit-plane weights"
                            ):
                                nc.vector.tensor_reduce(
                                    wpart[:, itl, ocol],
                                    scaled[:].rearrange("p (o n) -> p o n", n=NB),
                                    axis=mybir.AxisListType.X,
                                    op=mybir.AluOpType.add,
                                )

                    if mode not in ('dma',):
                        wp_dram = dram_pool.tile([P, N_ITL, OC], DT, tag="wp_dram")
                        wg_dram = dram_pool.tile([R, P, N_ITL, OC], DT, tag="wg_dram")
                        nc.gpsimd.dma_start(wp_dram[:], wpart[:])
                        nc.gpsimd.collective_compute(
                            "AllGather",
                            mybir.AluOpType.bypass,
                            replica_groups=[[0, 2, 4, 6], [1, 3, 5, 7]],
                            ins=[wp_dram.opt()],
                            outs=[wg_dram.opt()],
                        )

                if mode not in ('mm', 'dma'):
                    wT = wT_pool.tile([P, N_IT, OC], DT, tag="wT")
                    for tr in range(R):
                        for itl in range(N_ITL):
                            nc.sync.dma_start(
                                wT[:, tr * N_ITL + itl, :], wg_dram[tr, :, itl, :]
                            )
                elif mode == 'mm':
                    wT = wT_pool.tile([P, N_IT, OC], DT, tag="wT")

                # ---------------- main stage ----------------
                for sl in (range(0) if mode == 'w' else range(N_SLAB)):
                    tcols = slice(sl * TSLAB, (sl + 1) * TSLAB)
                    xs = xs_pool.tile([P, N_IT, TSLAB], DT, tag="xs")
                    for it in range(N_IT):
                        nc.sync.dma_start(
                            xs[:, it, :], xt_d[it * P : (it + 1) * P, tcols]
                        )
                    for v in range(VPS):
                        tt = sl * VPS + v
                        trow = slice(tt * P, (tt + 1) * P)
                        if mode == 'dma':
                            yt = yo_pool.tile([P, OC], DT, tag="yt")
                            nc.vector.tensor_copy(yt[:], bias_sb[:])
                            nc.scalar.dma_start(y_d[trow, :], yt[:])
                            continue
                        ps0 = mm_ps.tile([P, 512], mybir.dt.float32, tag="ps0")
                        ps1 = mm_ps.tile([P, 512], mybir.dt.float32, tag="ps1")
                        for it in range(N_IT):
                            lhsT = xs[:, it, v * P : (v + 1) * P]
                            nc.tensor.matmul(
                                ps0[:],
                                lhsT,
                                wT[:, it, 0:512],
                                start=(it == 0),
                                stop=(it == N_IT - 1),
                            )
                            nc.tensor.matmul(
                                ps1[:],
                                lhsT,
                                wT[:, it, 512:1024],
                                start=(it == 0),
                                stop=(it == N_IT - 1),
                            )
                        yt = yo_pool.tile([P, OC], DT, tag="yt")
                        nc.vector.tensor_tensor(
                            yt[:, 0:512], ps0[:], bias_sb[:, 0:512], mybir.AluOpType.add
                        )
                        nc.vector.tensor_tensor(
                            yt[:, 512:1024],
                            ps1[:],
                            bias_sb[:, 512:1024],
                            mybir.AluOpType.add,
                        )
                        nc.scalar.dma_start(y_d[trow, :], yt[:])

    nc.compile()
    return nc


def get_built():
    global _BUILT
    if _BUILT is None:
        _BUILT = _build_bass()
    return _BUILT


def make_in_maps(
    input, pweight, nweight, exps, bexps, mask_weight, scale, pbias, nbias, biasscale
):
    import ml_dtypes

    bf16 = ml_dtypes.bfloat16
    input = np.asarray(input, dtype=np.float32)
    pweight = np.asarray(pweight, dtype=np.float32)
    nweight = np.asarray(nweight, dtype=np.float32)
    exps = np.asarray(exps, dtype=np.float32)
    bexps = np.asarray(bexps, dtype=np.float32)
    mask_weight = np.asarray(mask_weight, dtype=np.float32)
    scale = np.asarray(scale, dtype=np.float32)
    pbias = np.asarray(pbias, dtype=np.float32)
    nbias = np.asarray(nbias, dtype=np.float32)
    biasscale = np.asarray(biasscale, dtype=np.float32)

    # tiny launch constants, computed exactly as the reference does
    mask = 1.0 / (1.0 + np.exp(-mask_weight))
    x = input.reshape(T, I)
    c4 = (exps * mask * scale[0]).astype(np.float32)  # [4]
    cvec = np.ascontiguousarray(np.broadcast_to(c4, (P, NB)).astype(bf16))

    bias_raw = (pbias - nbias) @ bexps  # [O]
    step = float(2**NB - 1)
    b = np.clip(bias_raw, -1.0, 1.0)
    bias = (np.round(np.abs(b) * step) / step * np.sign(b)) * biasscale[0]
    bias = bias.astype(np.float32)

    # [O, I, N] -> [I, O, N] per o-half, flattened (o, n); bf16
    pwT = [
        np.ascontiguousarray(
            pweight[oc * OC : (oc + 1) * OC].transpose(1, 0, 2)
        ).reshape(I, WCOL).astype(bf16)
        for oc in range(C)
    ]
    nwT = [
        np.ascontiguousarray(
            nweight[oc * OC : (oc + 1) * OC].transpose(1, 0, 2)
        ).reshape(I, WCOL).astype(bf16)
        for oc in range(C)
    ]

    in_maps = []
    for core in range(N_CORES):
        tr, oc = divmod(core, C)
        osl = slice(oc * OC, (oc + 1) * OC)
        isl = slice(tr * ISH, (tr + 1) * ISH)
        in_maps.append(
            {
                "xt": np.ascontiguousarray(x[tr * TQ : (tr + 1) * TQ].T.astype(bf16)),
                "pw": np.ascontiguousarray(pwT[oc][isl]),
                "nw": np.ascontiguousarray(nwT[oc][isl]),
                "cvec": cvec,
                "bias": np.ascontiguousarray(np.broadcast_to(bias[osl], (P, OC))),
            }
        )
    return in_maps


def gather_output(results):
    y = np.empty((T, O), dtype=np.float32)
    for core, r in enumerate(results):
        tr, oc = divmod(core, C)
        y[tr * TQ : (tr + 1) * TQ, oc * OC : (oc + 1) * OC] = np.asarray(
            r["y"]
        ).astype(np.float32)
    return y.reshape(8, T // 8, O)


def kernel(**inputs) -> np.ndarray:
    in_maps = make_in_maps(**inputs)
    nc = get_built()
    res = run_bass_kernel_spmd(nc, in_maps, core_ids=list(range(N_CORES)))
    return gather_output(res.results)


# revision 9
# speedup vs baseline: 2.8688x; 2.8688x over previous
"""BitLinear TRN2 kernel: y = x @ W(pweight,nweight)^T + bias.

Sharding: 8 cores = 4 token-shards x 2 out-feature-shards.
Per core: xt [2048 i, 4096 t] bf16 (token slice, host-transposed + cast),
pwt/nwt [512 i, 1024*4 (o,n)] bf16 (host-transposed so i is the partition
dim on device -> no PE transposes in weight prep).

Device pipeline (bf16 wire dtypes, fp32 PSUM accumulation):
  weights: DMA pwt/nwt -> ACT sigmoid -> DVE subtract -> DVE scale by
           c[n]=exps[n]*sigmoid(mask[n])*scale -> DVE reduce over n
           -> wpart [128, 4, 1024] bf16 -> AllGather over the 4 token
           shards (i-slices) -> wT [128 ip, 16 it, 1024 o] in SBUF.
  main:    per 128-token tile: lhsT = x-tile [128 i, 128 t] stationary,
           psum[t, o] += lhsT.T @ wT-slice over 16 i-tiles x 2 psum banks;
           DVE adds bias (host-replicated [128, 1024] tile) during
           PSUM->SBUF copy (bf16 out); DMA out y [t, o] bf16.

bias path: bit_ste is an exact identity on the reference's bias_raw values
(k/15 grid), computed host-side along with the tiny c vector.
"""

import numpy as np

import concourse.bass as bass
import concourse.mybir as mybir
import concourse.tile as tile
from concourse import bacc
from concourse.bass_utils import run_bass_kernel_spmd

N_CORES = 8
R, C = 4, 2  # token shards x out-feature shards
T, I, O, NB = 16384, 2048, 2048, 4
TQ, OC = T // R, O // C  # 4096 tokens, 1024 outs per core
P = 128
N_IT = I // P  # 16 i-tiles
ISH = I // R  # 512 i-rows of weight prep done locally per core
N_ITL = ISH // P  # 4 local i-tiles
WCOL = OC * NB  # 4096 flattened (o, n) columns of pwt/nwt
TSLAB = 512  # tokens per x slab (4 t-tiles)
N_SLAB = TQ // TSLAB
VPS = TSLAB // P  # t-tiles per slab
DT = mybir.dt.bfloat16

_BUILT = None


def _build_bass(reps=1, mode='full'):
    nc = bacc.Bacc("TRN2", debug=False, num_devices=N_CORES)

    xt_d = nc.dram_tensor("xt", [I, TQ], DT, kind="ExternalInput").ap()
    pw_d = nc.dram_tensor("pw", [ISH, WCOL], DT, kind="ExternalInput").ap()
    nw_d = nc.dram_tensor("nw", [ISH, WCOL], DT, kind="ExternalInput").ap()
    cv_d = nc.dram_tensor("cvec", [P, NB], DT, kind="ExternalInput").ap()
    bias_d = nc.dram_tensor("bias", [P, OC], mybir.dt.float32, kind="ExternalInput").ap()
    y_d = nc.dram_tensor("y", [TQ, OC], DT, kind="ExternalOutput").ap()

    with tile.TileContext(nc) as tc:
        with (
            tc.tile_pool(name="const", bufs=1) as const_pool,
            tc.tile_pool(name="wT", bufs=2) as wT_pool,
            tc.tile_pool(name="wpart", bufs=2) as wpart_pool,
            tc.tile_pool(name="dram", bufs=2, space="DRAM") as dram_pool,
            tc.tile_pool(name="wio", bufs=2) as wio_pool,
            tc.tile_pool(name="sig", bufs=2) as sig_pool,
            tc.tile_pool(name="soft", bufs=2) as soft_pool,
            tc.tile_pool(name="xs", bufs=2) as xs_pool,
            tc.tile_pool(name="yo", bufs=3) as yo_pool,
            tc.tile_pool(name="mm_ps", bufs=3, space="PSUM") as mm_ps,
        ):
            cv_sb = const_pool.tile([P, NB], DT)
            nc.sync.dma_start(cv_sb[:], cv_d[:])
            bias_sb = const_pool.tile([P, OC], mybir.dt.float32)
            nc.sync.dma_start(bias_sb[:], bias_d[:])

            for _rep in range(reps):
                # ---------------- weight stage (local i-slice) ----------------
                if mode != 'mm':
                    wpart = wpart_pool.tile([P, N_ITL, OC], DT, tag="wpart")
                    for itl in range(N_ITL):
                        irow = slice(itl * P, (itl + 1) * P)
                        for oh in range(2):  # half of (o, n) columns at a time
                            fcol = slice(oh * (WCOL // 2), (oh + 1) * (WCOL // 2))
                            ocol = slice(oh * (OC // 2), (oh + 1) * (OC // 2))
                            pwt = wio_pool.tile([P, WCOL // 2], DT, tag="pw")
                            nc.scalar.dma_start(pwt[:], pw_d[irow, fcol])
                            nwt = wio_pool.tile([P, WCOL // 2], DT, tag="nw")
                            nc.scalar.dma_start(nwt[:], nw_d[irow, fcol])
                            if mode == 'dma':
                                continue
                            sp = sig_pool.tile([P, WCOL // 2], DT, tag="sp")
                            nc.scalar.activation(
                                sp[:], pwt[:], mybir.ActivationFunctionType.Sigmoid
                            )
                            sn = sig_pool.tile([P, WCOL // 2], DT, tag="sn")
                            nc.scalar.activation(
                                sn[:], nwt[:], mybir.ActivationFunctionType.Sigmoid
                            )
                            soft = soft_pool.tile([P, WCOL // 2], DT, tag="soft")
                            nc.vector.tensor_sub(out=soft[:], in0=sp[:], in1=sn[:])
                            # scaled[i, o, n] = soft * c[n]; wpart[i, o] = sum_n
                            scaled = soft_pool.tile([P, WCOL // 2], DT, tag="scl")
                            nc.vector.tensor_tensor(
                                scaled[:].rearrange("p (o n) -> p o n", n=NB),
                                soft[:].rearrange("p (o n) -> p o n", n=NB),
                                cv_sb[:, None, :].to_broadcast((P, OC // 2, NB)),
                                mybir.AluOpType.mult,
                            )
                            with nc.allow_low_precision(
                                reason="4-term bf16 sum of bounded bit-plane weights"
                            ):
                                nc.vector.tensor_reduce(
                                    wpart[:, itl, ocol],
                                    scaled[:].rearrange("p (o n) -> p o n", n=NB),
                                    axis=mybir.AxisListType.X,
                                    op=mybir.AluOpType.add,
                                )

                    if mode not in ('dma',):
                        wp_dram = dram_pool.tile([P, N_ITL, OC], DT, tag="wp_dram")
                        wg_dram = dram_pool.tile([R, P, N_ITL, OC], DT, tag="wg_dram")
                        nc.gpsimd.dma_start(wp_dram[:], wpart[:])
                        nc.gpsimd.collective_compute(
                            "AllGather",
                            mybir.AluOpType.bypass,
                            replica_groups=[[0, 2, 4, 6], [1, 3, 5, 7]],
                            ins=[wp_dram.opt()],
                            outs=[wg_dram.opt()],
                        )

                if mode not in ('mm', 'dma'):
                    wT = wT_pool.tile([P, N_IT, OC], DT, tag="wT")
                    for tr in range(R):
                        for itl in range(N_ITL):
                            nc.sync.dma_start(
                                wT[:, tr * N_ITL + itl, :], wg_dram[tr, :, itl, :]
                            )
                elif mode == 'mm':
                    wT = wT_pool.tile([P, N_IT, OC], DT, tag="wT")
                    if _rep < 2:  # cover both pool buffers
                        nc.vector.memset(wT[:], 0.0)

                # ---------------- main stage ----------------
                for sl in (range(0) if mode == 'w' else range(N_SLAB)):
                    tcols = slice(sl * TSLAB, (sl + 1) * TSLAB)
                    xs = xs_pool.tile([P, N_IT, TSLAB], DT, tag="xs")
                    for it in range(N_IT):
                        nc.sync.dma_start(
                            xs[:, it, :], xt_d[it * P : (it + 1) * P, tcols]
                        )
                    for v in range(VPS):
                        tt = sl * VPS + v
                        trow = slice(tt * P, (tt + 1) * P)
                        if mode == 'dma':
                            yt = yo_pool.tile([P, OC], DT, tag="yt")
                            nc.vector.tensor_copy(yt[:], bias_sb[:])
                            nc.scalar.dma_start(y_d[trow, :], yt[:])
                            continue
                        ps0 = mm_ps.tile([P, 512], mybir.dt.float32, tag="ps0")
                        ps1 = mm_ps.tile([P, 512], mybir.dt.float32, tag="ps1")
                        for it in range(N_IT):
                            lhsT = xs[:, it, v * P : (v + 1) * P]
                            nc.tensor.matmul(
                                ps0[:],
                                lhsT,
                                wT[:, it, 0:512],
                                start=(it == 0),
                                stop=(it == N_IT - 1),
                            )
                            nc.tensor.matmul(
                                ps1[:],
                                lhsT,
                                wT[:, it, 512:1024],
                                start=(it == 0),
                                stop=(it == N_IT - 1),
                            )
                        yt = yo_pool.tile([P, OC], DT, tag="yt")
                        nc.vector.tensor_tensor(
                            yt[:, 0:512], ps0[:], bias_sb[:, 0:512], mybir.AluOpType.add
                        )
                        nc.vector.tensor_tensor(
                            yt[:, 512:1024],
                            ps1[:],
                            bias_sb[:, 512:1024],
                            mybir.AluOpType.add,
                        )
                        nc.scalar.dma_start(y_d[trow, :], yt[:])

    nc.compile()
    return nc


def get_built():
    global _BUILT
    if _BUILT is None:
        _BUILT = _build_bass()
    return _BUILT


def make_in_maps(
    input, pweight, nweight, exps, bexps, mask_weight, scale, pbias, nbias, biasscale
):
    import ml_dtypes

    bf16 = ml_dtypes.bfloat16
    input = np.asarray(input, dtype=np.float32)
    pweight = np.asarray(pweight, dtype=np.float32)
    nweight = np.asarray(nweight, dtype=np.float32)
    exps = np.asarray(exps, dtype=np.float32)
    bexps = np.asarray(bexps, dtype=np.float32)
    mask_weight = np.asarray(mask_weight, dtype=np.float32)
    scale = np.asarray(scale, dtype=np.float32)
    pbias = np.asarray(pbias, dtype=np.float32)
    nbias = np.asarray(nbias, dtype=np.float32)
    biasscale = np.asarray(biasscale, dtype=np.float32)

    # tiny launch constants, computed exactly as the reference does
    mask = 1.0 / (1.0 + np.exp(-mask_weight))
    x = input.reshape(T, I)
    c4 = (exps * mask * scale[0]).astype(np.float32)  # [4]
    cvec = np.ascontiguousarray(np.broadcast_to(c4, (P, NB)).astype(bf16))

    bias_raw = (pbias - nbias) @ bexps  # [O]
    step = float(2**NB - 1)
    b = np.clip(bias_raw, -1.0, 1.0)
    bias = (np.round(np.abs(b) * step) / step * np.sign(b)) * biasscale[0]
    bias = bias.astype(np.float32)

    # [O, I, N] -> [I, O, N] per o-half, flattened (o, n); bf16
    pwT = [
        np.ascontiguousarray(
            pweight[oc * OC : (oc + 1) * OC].transpose(1, 0, 2)
        ).reshape(I, WCOL).astype(bf16)
        for oc in range(C)
    ]
    nwT = [
        np.ascontiguousarray(
            nweight[oc * OC : (oc + 1) * OC].transpose(1, 0, 2)
        ).reshape(I, WCOL).astype(bf16)
        for oc in range(C)
    ]

    in_maps = []
    for core in range(N_CORES):
        tr, oc = divmod(core, C)
        osl = slice(oc * OC, (oc + 1) * OC)
        isl = slice(tr * ISH, (tr + 1) * ISH)
        in_maps.append(
            {
                "xt": np.ascontiguousarray(x[tr * TQ : (tr + 1) * TQ].T.astype(bf16)),
                "pw": np.ascontiguousarray(pwT[oc][isl]),
                "nw": np.ascontiguousarray(nwT[oc][isl]),
                "cvec": cvec,
                "bias": np.ascontiguousarray(np.broadcast_to(bias[osl], (P, OC))),
            }
        )
    return in_maps


def gather_output(results):
    y = np.empty((T, O), dtype=np.float32)
    for core, r in enumerate(results):
        tr, oc = divmod(core, C)
        y[tr * TQ : (tr + 1) * TQ, oc * OC : (oc + 1) * OC] = np.asarray(
            r["y"]
        ).astype(np.float32)
    return y.reshape(8, T // 8, O)


def kernel(**inputs) -> np.ndarray:
    in_maps = make_in_maps(**inputs)
    nc = get_built()
    res = run_bass_kernel_spmd(nc, in_maps, core_ids=list(range(N_CORES)))
    return gather_output(res.results)
